# revision 3
# baseline (speedup 1.0000x reference)
"""Trainium2 Bass kernel v2: single-head attention encoder block.

x[4, 2048, 1024]; q/k/v projections, softmax attention, output projection,
layernorm.  8 NeuronCores, SPMD: core c = (batch c//2, query-half c%2).
Host supplies each core's x ROLLED (so its queries are rows 0:1024) in BOTH
layouts: xb [S, D] and xt = xb^T [D, S] — no on-device transposes.

Per-core dataflow (fp32r matmuls except the bf16 value path):
  K^T  = Wk^T @ x          ([k, s], 8-bank PSUM accumulation, d-outer)
  Q^T  = Wq^T @ x[:1024]   ([k, q], wq streamed in k-halves)
  S^T  = K Q^T -> exp      ([s, q] bf16 unnormalized attn; den via DVE adds
                            + ones-matmul partition reduce)
  Z^T  = x^T @ exp(S^T)    ([d, q]; bf16 x tiles staged via ACT-queue DMA +
                            Pool cast)
  ctxT = (Wv^T @ Z^T)/den  ([v, q])
  h    = ctx @ Wo          ([q, d])
  out  = layernorm(h)*gamma + beta

LayerNorm touches ONLY the natural_log_exp activation table (exp/ln/copy/
square): sums come free from activation accum_out during the h drain, and
rstd = exp(-0.5*ln(var+eps)) — no Sqrt, so no act-table swaps anywhere.

Two independent HWDGE queues: SP carries weights/xt01/output, ACT carries
xt23 + the xb value stream, so streams never queue behind weight loads.
Epilogue work is spread across ACT (drains+stats), DVE (apply), Pool
(gamma/beta) so no engine exceeds the PE time per tile.
"""

from contextlib import ExitStack

import numpy as np

import concourse.bass as bass
import concourse.tile as tile
from concourse import bacc, mybir
from concourse.bass_utils import run_bass_kernel_spmd

F32 = mybir.dt.float32
F32R = mybir.dt.float32r
BF16 = mybir.dt.bfloat16
AF = mybir.ActivationFunctionType
OP = mybir.AluOpType

B = 4
S = 2048
D = 1024
NQ = 1024
P = 128
DT = D // P    # 8
ST = S // P    # 16
KTN = D // P   # 8
QTN = NQ // P  # 8
NC = 512
SCN = S // NC   # 4
QCN = NQ // NC  # 2
DCN = D // NC   # 2
N_CORES = 8
SCALE = 1.0 / np.sqrt(np.float32(D))
LN_EPS = 1e-5
RD = float(1.0 / D)


def _f32(ap):
    return ap.bitcast(F32)


def _keepalive(nc, tc, aps, out):
    kp = tc.alloc_tile_pool(name="keep", bufs=1, side="left")
    kt = kp.tile([P, max(len(aps), 1)], F32, tag="keep", name="keept")
    for i, ap in enumerate(aps):
        src_ap = ap[:, 0:1]
        if src_ap.dtype == F32R:
            src_ap = src_ap.bitcast(F32)
        nc.vector.tensor_copy(kt[:, i:i + 1], src_ap)
    nc.sync.dma_start(out[0:P, 0:max(len(aps), 1)], kt[:])
    kp.release()


def _emit(ctx: ExitStack, tc: tile.TileContext, io: dict, upto: str = "full"):
    nc = tc.nc
    xb = io["xb"]          # [S, D] f32r
    xt = io["xt"]          # [D, S] f32r
    wq = io["wq"]
    wk = io["wk"]
    wv = io["wv"]
    wo = io["wo"]
    gamma_b = io["gamma_b"]
    beta_b = io["beta_b"]
    out = io["out"]

    const = ctx.enter_context(tc.tile_pool(name="const", bufs=1, side="left"))
    ones_f = const.tile([P, P], F32, tag="ones_f")
    nc.vector.memset(ones_f[:], 1.0)
    ones = const.tile([P, P], F32R, tag="ones")
    nc.vector.tensor_copy(ones[:], ones_f[:])
    recip = const.tile([P, NQ], F32, tag="recip")
    eps_sb = const.tile([P, 1], F32, tag="eps")
    nc.vector.memset(eps_sb[:], LN_EPS)
    sq_scr = const.tile([P, NC], F32, tag="sqscr")

    # ---- SBUF stacks ----
    # left:  const | xt01 | wqs | wk | xt23, then S/C1: at|stage|xq0|xq1|den,
    #        then C2/O: wo | gb | h | o | stats
    # right: kt | qt, then ctxT | zt | wv
    xt01_pool = tc.alloc_tile_pool(name="xt01", bufs=1, side="left")
    xt01_all = xt01_pool.tile([P, DT * NQ], F32R, tag="xt01", name="xt01_all")
    xt01_v = [xt01_all[:, d * NQ:(d + 1) * NQ] for d in range(DT)]
    wqs_pool = tc.alloc_tile_pool(name="wqs", bufs=3, side="left")
    wk_pool = tc.alloc_tile_pool(name="wkp", bufs=1, side="left")
    wk_all = wk_pool.tile([P, DT * D], F32R, tag="wk", name="wk_all")
    wk_v = [wk_all[:, d * D:(d + 1) * D] for d in range(DT)]
    xt23_pool = tc.alloc_tile_pool(name="xt23", bufs=1, side="left")
    kt_pool = tc.alloc_tile_pool(name="ktp", bufs=1, side="right")
    kt_sb = [kt_pool.tile([P, S], F32R, tag=f"kt{k}", name=f"kt{k}") for k in range(KTN)]
    qt_pool = tc.alloc_tile_pool(name="qtp", bufs=1, side="right")
    qt_sb = [qt_pool.tile([P, NQ], F32R, tag=f"qt{k}", name=f"qt{k}") for k in range(KTN)]

    ps_a = tc.alloc_tile_pool(name="psA", bufs=1, space="PSUM")

    # DMA order.  SP queue carries every input (SP.SEQ runs ahead of the
    # compute engines, so next-iteration inputs start landing as soon as
    # their SBUF region frees at C1 end).  ACT queue carries only the
    # output tiles, so end-of-iteration waits never block SP.SEQ.
    # interleaved per-d DMAs: each (wk[d], xt01[d]) pair unlocks the next
    # d-step of the K phase, so PE starts after ~1 MB instead of 8 MB
    for d in range(DT):
        nc.sync.dma_start(wk_v[d], wk[d * P:(d + 1) * P, :])
        nc.sync.dma_start(xt01_v[d], xt[d * P:(d + 1) * P, 0:NQ])

    # ---- Phase K: K^T = Wk^T @ x, 8 accumulating banks, d streams ----
    xt23_r = {}
    for sc in (2, 3):
        for d in range(DT):
            t = xt23_pool.tile([P, NC], F32R, tag=f"x23_{d}", name=f"x23_{d}_{sc}")
            nc.sync.dma_start(t[:], xt[d * P:(d + 1) * P, sc * NC:(sc + 1) * NC])
            xt23_r[(sc, d)] = t
    for sc in range(SCN):
        ps = [ps_a.tile([P, NC], F32, tag=f"bank{k}", name=f"psK{k}_{sc}")
              for k in range(KTN)]
        for d in range(DT):
            src = (xt01_all[:, d * NQ + sc * NC:d * NQ + (sc + 1) * NC] if sc < 2
                   else xt23_r[(sc, d)][:])
            for k in range(KTN):
                nc.tensor.matmul(
                    ps[k][:], wk_all[:, d * D + k * P:d * D + (k + 1) * P], src,
                    start=(d == 0), stop=(d == DT - 1),
                )
        for k in range(KTN):
            dst = kt_sb[k][:, sc * NC:(sc + 1) * NC]
            if k % 2 == 0:
                nc.vector.tensor_copy(dst, ps[k][:])
            else:
                nc.scalar.copy(dst, ps[k][:])
    xt23_pool.release()
    wk_pool.release()

    if upto == "K":
        _keepalive(nc, tc, [t[:, 0:1] for t in kt_sb], out)
        ps_a.release()
        qt_pool.release()
        kt_pool.release()
        wqs_pool.release()
        xt01_pool.release()
        return

    # ---- Phase Q: Q^T = Wq^T @ x[:, :NQ], wq streamed in k-halves ----
    for kh in range(2):
        wq_h = []
        for d in range(DT):
            t = wqs_pool.tile([P, NC], F32R, tag="wqh", name=f"wqh{kh}_{d}")
            nc.sync.dma_start(t[:], wq[d * P:(d + 1) * P, kh * NC:(kh + 1) * NC])
            wq_h.append(t)
        ps = [ps_a.tile([P, NC], F32, tag=f"bank{i}", name=f"psQ{kh}_{i}")
              for i in range(KTN)]
        for d in range(DT):
            for qc in range(QCN):
                for kk in range(4):
                    nc.tensor.matmul(
                        ps[qc * 4 + kk][:],
                        wq_h[d][:, kk * P:(kk + 1) * P],
                        xt01_all[:, d * NQ + qc * NC:d * NQ + (qc + 1) * NC],
                        start=(d == 0), stop=(d == DT - 1),
                    )
        for qc in range(QCN):
            for kk in range(4):
                k = kh * 4 + kk
                dst = qt_sb[k][:, qc * NC:(qc + 1) * NC]
                if (qc * 4 + kk) % 2 == 0:
                    nc.vector.tensor_copy(dst, ps[qc * 4 + kk][:])
                else:
                    nc.scalar.copy(dst, ps[qc * 4 + kk][:])
    wqs_pool.release()
    xt01_pool.release()
    ps_a.release()

    if upto == "Q":
        _keepalive(nc, tc, [t[:, 0:1] for t in kt_sb] + [t[:, 0:1] for t in qt_sb], out)
        qt_pool.release()
        kt_pool.release()
        return

    # ---- Phase S: scores^T -> exp -> bf16 at, den; xb value stream starts ----
    at_pool = tc.alloc_tile_pool(name="atp", bufs=1, side="left")
    at_sb = [at_pool.tile([P, NQ], BF16, tag=f"at{st}", name=f"at{st}") for st in range(ST)]
    stage_pool = tc.alloc_tile_pool(name="stage", bufs=6, side="left")
    xq0_pool = tc.alloc_tile_pool(name="xq0", bufs=1, side="left")
    xq0 = [xq0_pool.tile([P, NC], BF16, tag=f"xq0_{st}", name=f"xq0_{st}")
           for st in range(ST)]
    xq1_pool = tc.alloc_tile_pool(name="xq1", bufs=1, side="left")
    xq1 = [xq1_pool.tile([P, NC], BF16, tag=f"xq1_{st}", name=f"xq1_{st}")
           for st in range(ST)]
    den_pool = tc.alloc_tile_pool(name="denp", bufs=2, side="left")
    ps_b = tc.alloc_tile_pool(name="psB", bufs=7, space="PSUM")
    ps_d = tc.alloc_tile_pool(name="psD", bufs=1, space="PSUM")

    # xb stream: DMA fp32 -> stage (ACT queue), Pool casts to bf16 tiles.
    for dp in range(2):
        for st in range(ST):
            stg = stage_pool.tile([P, NC], F32R, tag="stg", name=f"stg{dp}_{st}")
            nc.sync.dma_start(stg[:], xb[st * P:(st + 1) * P, dp * NC:(dp + 1) * NC])
            dst = (xq0 if dp == 0 else xq1)[st]
            nc.gpsimd.tensor_copy(dst[:], _f32(stg[:]))

    for qc in range(QCN):
        dsb = den_pool.tile([P, NC], F32, tag="densb", name=f"densb{qc}")
        nc.vector.memset(dsb[:], 0.0)
        for st in range(ST):
            ps = ps_b.tile([P, NC], F32, tag="mm", name=f"psS{qc}_{st}")
            for k in range(KTN):
                nc.tensor.matmul(
                    ps[:], kt_sb[k][:, st * P:(st + 1) * P],
                    qt_sb[k][:, qc * NC:(qc + 1) * NC],
                    start=(k == 0), stop=(k == KTN - 1),
                )
            nc.scalar.activation(
                at_sb[st][:, qc * NC:(qc + 1) * NC], ps[:], AF.Exp, scale=float(SCALE)
            )
            nc.vector.tensor_tensor(
                dsb[:], dsb[:], at_sb[st][:, qc * NC:(qc + 1) * NC], OP.add
            )
        dsr = den_pool.tile([P, NC], F32R, tag="densr", name=f"densr{qc}")
        nc.vector.tensor_copy(dsr[:], dsb[:])
        dps = ps_d.tile([P, NC], F32, tag="den", name=f"dps{qc}")
        nc.tensor.matmul(dps[:], ones[:], dsr[:], start=True, stop=True)
        nc.vector.reciprocal(recip[:, qc * NC:(qc + 1) * NC], dps[:])
    den_pool.release()
    qt_pool.release()
    kt_pool.release()

    if upto == "S":
        _keepalive(nc, tc, [t[:, 0:1] for t in at_sb] + [recip[:, 0:1]], out)
        xq1_pool.release()
        xq0_pool.release()
        stage_pool.release()
        at_pool.release()
        ps_d.release()
        ps_b.release()
        return

    # ---- Phase C1: Z^T = x^T @ exp(S^T), bf16 matmuls ----
    # The whole value path (at/xq/zt/wv/ctxT/wo) is bf16: same PE rate, and
    # the halved right-stack footprint keeps it clear of the next
    # iteration's left-stack input region, so input DMAs overlap C2/O.
    ctxT_pool = tc.alloc_tile_pool(name="ctxTp", bufs=1, side="right")
    ctxT = [ctxT_pool.tile([P, NQ], BF16, tag=f"cxT{v}", name=f"cxT{v}") for v in range(DT)]
    zt_pool = tc.alloc_tile_pool(name="ztp", bufs=1, side="right")
    zt_sb = [zt_pool.tile([P, NQ], BF16, tag=f"zt{d}", name=f"zt{d}") for d in range(DT)]
    wv_pool = tc.alloc_tile_pool(name="wvp", bufs=1, side="right")
    wv_sb = [wv_pool.tile([P, D], BF16, tag=f"wv{d}", name=f"wv{d}") for d in range(DT)]
    wstage_pool = tc.alloc_tile_pool(name="wstage", bufs=2, side="left")
    for d in range(DT):
        ws = wstage_pool.tile([P, D], F32R, tag="ws", name=f"wvs{d}")
        nc.sync.dma_start(ws[:], wv[d * P:(d + 1) * P, :])
        nc.gpsimd.tensor_copy(wv_sb[d][:], _f32(ws[:]))

    for dp in range(2):
        xq = xq0 if dp == 0 else xq1
        for dh in range(4):
            d = 4 * dp + dh
            for qc in range(QCN):
                ps = ps_b.tile([P, NC], F32, tag="mm", name=f"psZ{d}_{qc}")
                for st in range(ST):
                    nc.tensor.matmul(
                        ps[:], xq[st][:, dh * P:(dh + 1) * P],
                        at_sb[st][:, qc * NC:(qc + 1) * NC],
                        start=(st == 0), stop=(st == ST - 1),
                    )
                dst = zt_sb[d][:, qc * NC:(qc + 1) * NC]
                if d % 2 == 0:
                    nc.vector.tensor_copy(dst, ps[:])
                else:
                    nc.scalar.copy(dst, ps[:])
    wstage_pool.release()
    xq1_pool.release()
    xq0_pool.release()
    stage_pool.release()
    at_pool.release()

    if upto == "C1":
        _keepalive(
            nc, tc,
            [t[:, 0:1] for t in zt_sb] + [t[:, 0:1] for t in wv_sb] + [recip[:, 0:1]],
            out)
        wv_pool.release()
        zt_pool.release()
        ctxT_pool.release()
        ps_d.release()
        ps_b.release()
        return

    # ---- Phase C2: ctxT = (Wv^T @ Z^T) / den ----
    wo_pool = tc.alloc_tile_pool(name="wop", bufs=1, side="right")
    wo_sb = [wo_pool.tile([P, D], BF16, tag=f"wo{v}", name=f"wo{v}") for v in range(DT)]
    gb_pool = tc.alloc_tile_pool(name="gbp", bufs=1, side="right")
    gamma_sb = gb_pool.tile([P, D], F32, tag="gamma", name="gamma_sb")
    nc.sync.dma_start(gamma_sb[:], gamma_b[:])
    beta_sb = gb_pool.tile([P, D], F32, tag="beta", name="beta_sb")
    nc.sync.dma_start(beta_sb[:], beta_b[:])
    h_pool = tc.alloc_tile_pool(name="hp", bufs=3, side="right")
    stat_pool = tc.alloc_tile_pool(name="statp", bufs=4, side="right")
    # wo loads staged through h tiles (fp32), cast to bf16 by Pool
    for v in range(DT):
        ws = h_pool.tile([P, D], F32, tag="h", name=f"wos{v}")
        nc.sync.dma_start(ws[:].bitcast(F32R), wo[v * P:(v + 1) * P, :])
        nc.gpsimd.tensor_copy(wo_sb[v][:], ws[:])

    for vt in range(DT):
        for qc in range(QCN):
            ps = ps_b.tile([P, NC], F32, tag="mm", name=f"psC{vt}_{qc}")
            for d in range(DT):
                nc.tensor.matmul(
                    ps[:], wv_sb[d][:, vt * P:(vt + 1) * P],
                    zt_sb[d][:, qc * NC:(qc + 1) * NC],
                    start=(d == 0), stop=(d == DT - 1),
                )
            dst = ctxT[vt][:, qc * NC:(qc + 1) * NC]
            nc.vector.tensor_tensor(dst, ps[:], recip[:, qc * NC:(qc + 1) * NC], OP.mult)

    if upto == "C2":
        _keepalive(
            nc, tc,
            [t[:, 0:1] for t in ctxT] + [t[:, 0:1] for t in wo_sb]
            + [gamma_sb[:, 0:1], beta_sb[:, 0:1]], out)
        stat_pool.release()
        h_pool.release()
        gb_pool.release()
        wo_pool.release()
        wv_pool.release()
        zt_pool.release()
        ctxT_pool.release()
        ps_d.release()
        ps_b.release()
        return

    # ---- Phase O: h = ctx @ Wo, layernorm via accum_out + ln/exp rsqrt ----
    o_done = []
    for qt in range(QTN):
        h = h_pool.tile([P, D], F32, tag="h", name=f"h{qt}")
        sums = stat_pool.tile([P, 2], F32, tag="sums", name=f"sums{qt}")
        sqs = stat_pool.tile([P, 2], F32, tag="sqs", name=f"sqs{qt}")
        for dc in range(DCN):
            ps = ps_b.tile([P, NC], F32, tag="mm", name=f"psO{qt}_{dc}")
            for v in range(DT):
                nc.tensor.matmul(
                    ps[:], ctxT[v][:, qt * P:(qt + 1) * P],
                    wo_sb[v][:, dc * NC:(dc + 1) * NC],
                    start=(v == 0), stop=(v == DT - 1),
                )
            nc.scalar.activation(
                h[:, dc * NC:(dc + 1) * NC], ps[:], AF.Copy,
                accum_out=sums[:, dc:dc + 1],
            )
            nc.scalar.activation(
                sq_scr[:], ps[:], AF.Square, accum_out=sqs[:, dc:dc + 1],
            )
        mu = stat_pool.tile([P, 1], F32, tag="mu", name=f"mu{qt}")
        nc.vector.tensor_tensor(mu[:], sums[:, 0:1], sums[:, 1:2], OP.add)
        nc.vector.tensor_scalar(out=mu[:], in0=mu[:], scalar1=RD, scalar2=None, op0=OP.mult)
        m2 = stat_pool.tile([P, 1], F32, tag="m2", name=f"m2{qt}")
        nc.vector.tensor_tensor(m2[:], sqs[:, 0:1], sqs[:, 1:2], OP.add)
        nc.vector.tensor_scalar(out=m2[:], in0=m2[:], scalar1=RD, scalar2=None, op0=OP.mult)
        var = stat_pool.tile([P, 1], F32, tag="var", name=f"var{qt}")
        nc.vector.tensor_tensor(var[:], mu[:], mu[:], OP.mult)
        nc.vector.tensor_tensor(var[:], m2[:], var[:], OP.subtract)
        # rstd = exp(-0.5 * ln(var + eps)) — stays on the exp/ln act table
        lnv = stat_pool.tile([P, 1], F32, tag="lnv", name=f"lnv{qt}")
        nc.scalar.activation(lnv[:], var[:], AF.Ln, bias=eps_sb[:], scale=1.0)
        rstd = stat_pool.tile([P, 1], F32, tag="rstd", name=f"rstd{qt}")
        nc.scalar.activation(rstd[:], lnv[:], AF.Exp, scale=-0.5)
        # (h - mu)*gamma on DVE, then (*rstd) + beta on Pool, both in-place
        # in h: each engine appears once per tile so chains pipeline.
        nc.vector.scalar_tensor_tensor(
            out=h[:], in0=h[:], scalar=mu[:], in1=gamma_sb[:],
            op0=OP.subtract, op1=OP.mult,
        )
        nc.vector.scalar_tensor_tensor(
            out=h[:], in0=h[:], scalar=rstd[:], in1=beta_sb[:],
            op0=OP.mult, op1=OP.add,
        )
        o_done.append(h)
        if qt >= 2:
            q0 = qt - 2
            nc.scalar.dma_start(out[q0 * P:(q0 + 1) * P, :], o_done[q0][:])
    for q0 in (QTN - 2, QTN - 1):
        nc.scalar.dma_start(out[q0 * P:(q0 + 1) * P, :], o_done[q0][:])
    stat_pool.release()
    h_pool.release()
    gb_pool.release()
    wo_pool.release()
    wv_pool.release()
    zt_pool.release()
    ctxT_pool.release()
    ps_d.release()
    ps_b.release()


_PROGS: dict = {}

_ACT_KEEP = "natural_log_exp_and_others"


def _compile_single_act_table(nc):
    """Compile with every act-table set except _ACT_KEEP hidden, so the
    table-load pass places exactly one LoadActFuncSet (ids stay valid
    because the list order/indexes are unchanged)."""
    orig = bacc.get_activation_tables

    def filtered(arch):
        full = orig(arch)
        if _ACT_KEEP not in full:
            return full
        return {name: (funcs if name == _ACT_KEEP else set())
                for name, funcs in full.items()}

    bacc.get_activation_tables = filtered
    try:
        nc.compile()
    finally:
        bacc.get_activation_tables = orig


def _build_program(n_iters: int = 1, upto: str = "full"):
    key = (n_iters, upto)
    if key not in _PROGS:
        nc = bacc.Bacc(
            "TRN2",
            target_bir_lowering=False,
            debug=False,
            enable_asserts=False,
            num_devices=N_CORES,
        )
        io = {
            "xb": nc.dram_tensor("xb", [S, D], F32R, kind="ExternalInput").ap(),
            "xt": nc.dram_tensor("xt", [D, S], F32R, kind="ExternalInput").ap(),
            "wq": nc.dram_tensor("wq", [D, D], F32R, kind="ExternalInput").ap(),
            "wk": nc.dram_tensor("wk", [D, D], F32R, kind="ExternalInput").ap(),
            "wv": nc.dram_tensor("wv", [D, D], F32R, kind="ExternalInput").ap(),
            "wo": nc.dram_tensor("wo", [D, D], F32R, kind="ExternalInput").ap(),
            "gamma_b": nc.dram_tensor("gamma_b", [P, D], F32, kind="ExternalInput").ap(),
            "beta_b": nc.dram_tensor("beta_b", [P, D], F32, kind="ExternalInput").ap(),
            "out": nc.dram_tensor("out", [NQ, D], F32, kind="ExternalOutput").ap(),
        }
        with tile.TileContext(nc) as tc:
            for _ in range(n_iters):
                with ExitStack() as ctx:
                    _emit(ctx, tc, io, upto)
        _compile_single_act_table(nc)
        _PROGS[key] = nc
    return _PROGS[key]


LAST_RESULTS = None


def kernel(x, Wq, Wk, Wv, Wo, ln2_gamma, ln2_beta):
    global LAST_RESULTS
    x = np.ascontiguousarray(np.asarray(x, dtype=np.float32))
    Wq = np.ascontiguousarray(np.asarray(Wq, dtype=np.float32))
    Wk = np.ascontiguousarray(np.asarray(Wk, dtype=np.float32))
    Wv = np.ascontiguousarray(np.asarray(Wv, dtype=np.float32))
    Wo = np.ascontiguousarray(np.asarray(Wo, dtype=np.float32))
    gamma_b = np.ascontiguousarray(
        np.broadcast_to(np.asarray(ln2_gamma, dtype=np.float32), (P, D))
    )
    beta_b = np.ascontiguousarray(
        np.broadcast_to(np.asarray(ln2_beta, dtype=np.float32), (P, D))
    )

    nc = _build_program()
    in_maps = []
    for c in range(N_CORES):
        b, h = c // 2, c % 2
        xr = np.ascontiguousarray(np.roll(x[b], -h * NQ, axis=0))
        in_maps.append(
            {
                "xb": xr,
                "xt": np.ascontiguousarray(xr.T),
                "wq": Wq,
                "wk": Wk,
                "wv": Wv,
                "wo": Wo,
                "gamma_b": gamma_b,
                "beta_b": beta_b,
            }
        )
    res = run_bass_kernel_spmd(nc, in_maps, list(range(N_CORES)))
    LAST_RESULTS = res
    out = np.empty((B, S, D), dtype=np.float32)
    for c in range(N_CORES):
        b, h = c // 2, c % 2
        out[b, h * NQ:(h + 1) * NQ] = res.results[c]["out"]
    return out


# revision 5
# speedup vs baseline: 1.0953x; 1.0953x over previous
"""Trainium2 Bass kernel v2: single-head attention encoder block.

x[4, 2048, 1024]; q/k/v projections, softmax attention, output projection,
layernorm.  8 NeuronCores, SPMD: core c = (batch c//2, query-half c%2).
Host supplies each core's x ROLLED (so its queries are rows 0:1024) in BOTH
layouts: xb [S, D] and xt = xb^T [D, S] — no on-device transposes.

Per-core dataflow (fp32r matmuls except the bf16 value path):
  K^T  = Wk^T @ x          ([k, s], 8-bank PSUM accumulation, d-outer)
  Q^T  = Wq^T @ x[:1024]   ([k, q], wq streamed in k-halves)
  S^T  = K Q^T -> exp      ([s, q] bf16 UNNORMALIZED attn; the softmax
                            denominator is never computed — LayerNorm is
                            invariant to positive row scaling, so it cancels
                            (eps mismatch costs ~2e-3 relative))
  Z^T  = x^T @ exp(S^T)    ([d, q]; bf16 x tiles staged + Pool cast)
  ctxT = Wv^T @ Z^T        ([v, q], unnormalized)
  h    = ctx @ Wo          ([q, d])
  out  = layernorm(h)*gamma + beta

LayerNorm touches ONLY the natural_log_exp activation table (exp/ln/copy/
square): sums come free from activation accum_out during the h drain, and
rstd = exp(-0.5*ln(var+eps)) — no Sqrt, so no act-table swaps anywhere.

Two independent HWDGE queues: SP carries weights/xt01/output, ACT carries
xt23 + the xb value stream, so streams never queue behind weight loads.
Epilogue work is spread across ACT (drains+stats), DVE (apply), Pool
(gamma/beta) so no engine exceeds the PE time per tile.
"""

from contextlib import ExitStack

import numpy as np

import concourse.bass as bass
import concourse.tile as tile
from concourse import bacc, mybir
from concourse.bass_utils import run_bass_kernel_spmd

F32 = mybir.dt.float32
F32R = mybir.dt.float32r
BF16 = mybir.dt.bfloat16
AF = mybir.ActivationFunctionType
OP = mybir.AluOpType

B = 4
S = 2048
D = 1024
NQ = 1024
P = 128
DT = D // P    # 8
ST = S // P    # 16
KTN = D // P   # 8
QTN = NQ // P  # 8
NC = 512
SCN = S // NC   # 4
QCN = NQ // NC  # 2
DCN = D // NC   # 2
N_CORES = 8
SCALE = 1.0 / np.sqrt(np.float32(D))
LN_EPS = 1e-5
RD = float(1.0 / D)


def _f32(ap):
    return ap.bitcast(F32)


def _keepalive(nc, tc, aps, out):
    kp = tc.alloc_tile_pool(name="keep", bufs=1, side="left")
    kt = kp.tile([P, max(len(aps), 1)], F32, tag="keep", name="keept")
    for i, ap in enumerate(aps):
        src_ap = ap[:, 0:1]
        if src_ap.dtype == F32R:
            src_ap = src_ap.bitcast(F32)
        nc.vector.tensor_copy(kt[:, i:i + 1], src_ap)
    nc.sync.dma_start(out[0:P, 0:max(len(aps), 1)], kt[:])
    kp.release()


def _emit(ctx: ExitStack, tc: tile.TileContext, io: dict, upto: str = "full"):
    nc = tc.nc
    xb = io["xb"]          # [S, D] f32r
    xt = io["xt"]          # [D, S] f32r
    wq = io["wq"]
    wk = io["wk"]
    wv = io["wv"]
    wo = io["wo"]
    gamma_b = io["gamma_b"]
    beta_b = io["beta_b"]
    out = io["out"]

    const = ctx.enter_context(tc.tile_pool(name="const", bufs=1, side="left"))
    eps_sb = const.tile([P, 1], F32, tag="eps")
    nc.vector.memset(eps_sb[:], LN_EPS)
    sq_scr = const.tile([P, NC], F32, tag="sqscr")

    # ---- SBUF stacks ----
    # left:  const | xt01 | wqs | wk | xt23, then S/C1: at|stage|xq0|xq1|den,
    #        then C2/O: wo | gb | h | o | stats
    # right: kt | qt, then ctxT | zt | wv
    xt01_pool = tc.alloc_tile_pool(name="xt01", bufs=1, side="left")
    xt01_all = xt01_pool.tile([P, DT * NQ], F32R, tag="xt01", name="xt01_all")
    xt01_v = [xt01_all[:, d * NQ:(d + 1) * NQ] for d in range(DT)]
    wqs_pool = tc.alloc_tile_pool(name="wqs", bufs=3, side="left")
    wk_pool = tc.alloc_tile_pool(name="wkp", bufs=1, side="left")
    wk_all = wk_pool.tile([P, DT * D], F32R, tag="wk", name="wk_all")
    wk_v = [wk_all[:, d * D:(d + 1) * D] for d in range(DT)]
    xt23_pool = tc.alloc_tile_pool(name="xt23", bufs=1, side="left")
    kt_pool = tc.alloc_tile_pool(name="ktp", bufs=1, side="right")
    kt_sb = [kt_pool.tile([P, S], F32R, tag=f"kt{k}", name=f"kt{k}") for k in range(KTN)]
    qt_pool = tc.alloc_tile_pool(name="qtp", bufs=1, side="right")
    qt_sb = [qt_pool.tile([P, NQ], F32R, tag=f"qt{k}", name=f"qt{k}") for k in range(KTN)]

    ps_a = tc.alloc_tile_pool(name="psA", bufs=1, space="PSUM")

    # DMA order.  SP queue carries every input (SP.SEQ runs ahead of the
    # compute engines, so next-iteration inputs start landing as soon as
    # their SBUF region frees at C1 end).  ACT queue carries only the
    # output tiles, so end-of-iteration waits never block SP.SEQ.
    # interleaved per-d DMAs: each (wk[d], xt01[d]) pair unlocks the next
    # d-step of the K phase, so PE starts after ~1 MB instead of 8 MB
    for d in range(DT):
        nc.sync.dma_start(wk_v[d], wk[d * P:(d + 1) * P, :])
        nc.sync.dma_start(xt01_v[d], xt[d * P:(d + 1) * P, 0:NQ])

    # ---- Phase K: K^T = Wk^T @ x, 8 accumulating banks, d streams ----
    xt23_r = {}
    for sc in (2, 3):
        for d in range(DT):
            t = xt23_pool.tile([P, NC], F32R, tag=f"x23_{d}", name=f"x23_{d}_{sc}")
            nc.sync.dma_start(t[:], xt[d * P:(d + 1) * P, sc * NC:(sc + 1) * NC])
            xt23_r[(sc, d)] = t
    for sc in range(SCN):
        ps = [ps_a.tile([P, NC], F32, tag=f"bank{k}", name=f"psK{k}_{sc}")
              for k in range(KTN)]
        for d in range(DT):
            src = (xt01_all[:, d * NQ + sc * NC:d * NQ + (sc + 1) * NC] if sc < 2
                   else xt23_r[(sc, d)][:])
            for k in range(KTN):
                nc.tensor.matmul(
                    ps[k][:], wk_all[:, d * D + k * P:d * D + (k + 1) * P], src,
                    start=(d == 0), stop=(d == DT - 1),
                )
        for k in range(KTN):
            dst = kt_sb[k][:, sc * NC:(sc + 1) * NC]
            if k % 2 == 0:
                nc.vector.tensor_copy(dst, ps[k][:])
            else:
                nc.scalar.copy(dst, ps[k][:])
    xt23_pool.release()
    wk_pool.release()

    if upto == "K":
        _keepalive(nc, tc, [t[:, 0:1] for t in kt_sb], out)
        ps_a.release()
        qt_pool.release()
        kt_pool.release()
        wqs_pool.release()
        xt01_pool.release()
        return

    # ---- Phase Q: Q^T = Wq^T @ x[:, :NQ], wq streamed in k-halves ----
    for kh in range(2):
        wq_h = []
        for d in range(DT):
            t = wqs_pool.tile([P, NC], F32R, tag="wqh", name=f"wqh{kh}_{d}")
            nc.sync.dma_start(t[:], wq[d * P:(d + 1) * P, kh * NC:(kh + 1) * NC])
            wq_h.append(t)
        ps = [ps_a.tile([P, NC], F32, tag=f"bank{i}", name=f"psQ{kh}_{i}")
              for i in range(KTN)]
        for d in range(DT):
            for qc in range(QCN):
                for kk in range(4):
                    nc.tensor.matmul(
                        ps[qc * 4 + kk][:],
                        wq_h[d][:, kk * P:(kk + 1) * P],
                        xt01_all[:, d * NQ + qc * NC:d * NQ + (qc + 1) * NC],
                        start=(d == 0), stop=(d == DT - 1),
                    )
        for qc in range(QCN):
            for kk in range(4):
                k = kh * 4 + kk
                dst = qt_sb[k][:, qc * NC:(qc + 1) * NC]
                if (qc * 4 + kk) % 2 == 0:
                    nc.vector.tensor_copy(dst, ps[qc * 4 + kk][:])
                else:
                    nc.scalar.copy(dst, ps[qc * 4 + kk][:])
    wqs_pool.release()
    xt01_pool.release()
    ps_a.release()

    if upto == "Q":
        _keepalive(nc, tc, [t[:, 0:1] for t in kt_sb] + [t[:, 0:1] for t in qt_sb], out)
        qt_pool.release()
        kt_pool.release()
        return

    # ---- Phase S: scores^T -> exp -> bf16 at; xb value stream starts ----
    at_pool = tc.alloc_tile_pool(name="atp", bufs=1, side="left")
    at_sb = [at_pool.tile([P, NQ], BF16, tag=f"at{st}", name=f"at{st}") for st in range(ST)]
    stage_pool = tc.alloc_tile_pool(name="stage", bufs=6, side="left")
    xq0_pool = tc.alloc_tile_pool(name="xq0", bufs=1, side="left")
    xq0 = [xq0_pool.tile([P, NC], BF16, tag=f"xq0_{st}", name=f"xq0_{st}")
           for st in range(ST)]
    xq1_pool = tc.alloc_tile_pool(name="xq1", bufs=1, side="left")
    xq1 = [xq1_pool.tile([P, NC], BF16, tag=f"xq1_{st}", name=f"xq1_{st}")
           for st in range(ST)]
    ps_b = tc.alloc_tile_pool(name="psB", bufs=7, space="PSUM")

    # xb stream: DMA fp32 -> stage (ACT queue), Pool casts to bf16 tiles.
    for dp in range(2):
        for st in range(ST):
            stg = stage_pool.tile([P, NC], F32R, tag="stg", name=f"stg{dp}_{st}")
            nc.sync.dma_start(stg[:], xb[st * P:(st + 1) * P, dp * NC:(dp + 1) * NC])
            dst = (xq0 if dp == 0 else xq1)[st]
            nc.gpsimd.tensor_copy(dst[:], _f32(stg[:]))

    for qc in range(QCN):
        for st in range(ST):
            ps = ps_b.tile([P, NC], F32, tag="mm", name=f"psS{qc}_{st}")
            for k in range(KTN):
                nc.tensor.matmul(
                    ps[:], kt_sb[k][:, st * P:(st + 1) * P],
                    qt_sb[k][:, qc * NC:(qc + 1) * NC],
                    start=(k == 0), stop=(k == KTN - 1),
                )
            nc.scalar.activation(
                at_sb[st][:, qc * NC:(qc + 1) * NC], ps[:], AF.Exp, scale=float(SCALE)
            )
    qt_pool.release()
    kt_pool.release()

    if upto == "S":
        _keepalive(nc, tc, [t[:, 0:1] for t in at_sb], out)
        xq1_pool.release()
        xq0_pool.release()
        stage_pool.release()
        at_pool.release()
        ps_b.release()
        return

    # ---- Phase C1: Z^T = x^T @ exp(S^T), bf16 matmuls ----
    # The whole value path (at/xq/zt/wv/ctxT/wo) is bf16: same PE rate, and
    # the halved right-stack footprint keeps it clear of the next
    # iteration's left-stack input region, so input DMAs overlap C2/O.
    ctxT_pool = tc.alloc_tile_pool(name="ctxTp", bufs=1, side="right")
    ctxT = [ctxT_pool.tile([P, NQ], BF16, tag=f"cxT{v}", name=f"cxT{v}") for v in range(DT)]
    zt_pool = tc.alloc_tile_pool(name="ztp", bufs=1, side="right")
    zt_sb = [zt_pool.tile([P, NQ], BF16, tag=f"zt{d}", name=f"zt{d}") for d in range(DT)]
    wv_pool = tc.alloc_tile_pool(name="wvp", bufs=1, side="right")
    wv_sb = [wv_pool.tile([P, D], BF16, tag=f"wv{d}", name=f"wv{d}") for d in range(DT)]
    wstage_pool = tc.alloc_tile_pool(name="wstage", bufs=2, side="left")
    for d in range(DT):
        ws = wstage_pool.tile([P, D], F32R, tag="ws", name=f"wvs{d}")
        nc.sync.dma_start(ws[:], wv[d * P:(d + 1) * P, :])
        nc.gpsimd.tensor_copy(wv_sb[d][:], _f32(ws[:]))

    for dp in range(2):
        xq = xq0 if dp == 0 else xq1
        for dh in range(4):
            d = 4 * dp + dh
            for qc in range(QCN):
                ps = ps_b.tile([P, NC], F32, tag="mm", name=f"psZ{d}_{qc}")
                for st in range(ST):
                    nc.tensor.matmul(
                        ps[:], xq[st][:, dh * P:(dh + 1) * P],
                        at_sb[st][:, qc * NC:(qc + 1) * NC],
                        start=(st == 0), stop=(st == ST - 1),
                    )
                dst = zt_sb[d][:, qc * NC:(qc + 1) * NC]
                if d % 2 == 0:
                    nc.vector.tensor_copy(dst, ps[:])
                else:
                    nc.scalar.copy(dst, ps[:])
    wstage_pool.release()
    xq1_pool.release()
    xq0_pool.release()
    stage_pool.release()
    at_pool.release()

    if upto == "C1":
        _keepalive(
            nc, tc,
            [t[:, 0:1] for t in zt_sb] + [t[:, 0:1] for t in wv_sb],
            out)
        wv_pool.release()
        zt_pool.release()
        ctxT_pool.release()
        ps_b.release()
        return

    # ---- Phase C2: ctxT = Wv^T @ Z^T (unnormalized) ----
    wo_pool = tc.alloc_tile_pool(name="wop", bufs=1, side="right")
    wo_sb = [wo_pool.tile([P, D], BF16, tag=f"wo{v}", name=f"wo{v}") for v in range(DT)]
    gb_pool = tc.alloc_tile_pool(name="gbp", bufs=1, side="right")
    gamma_sb = gb_pool.tile([P, D], F32, tag="gamma", name="gamma_sb")
    nc.sync.dma_start(gamma_sb[:], gamma_b[:])
    beta_sb = gb_pool.tile([P, D], F32, tag="beta", name="beta_sb")
    nc.sync.dma_start(beta_sb[:], beta_b[:])
    h_pool = tc.alloc_tile_pool(name="hp", bufs=3, side="right")
    stat_pool = tc.alloc_tile_pool(name="statp", bufs=4, side="right")
    # wo loads staged through h tiles (fp32), cast to bf16 by Pool
    for v in range(DT):
        ws = h_pool.tile([P, D], F32, tag="h", name=f"wos{v}")
        nc.sync.dma_start(ws[:].bitcast(F32R), wo[v * P:(v + 1) * P, :])
        nc.gpsimd.tensor_copy(wo_sb[v][:], ws[:])

    for vt in range(DT):
        for qc in range(QCN):
            ps = ps_b.tile([P, NC], F32, tag="mm", name=f"psC{vt}_{qc}")
            for d in range(DT):
                nc.tensor.matmul(
                    ps[:], wv_sb[d][:, vt * P:(vt + 1) * P],
                    zt_sb[d][:, qc * NC:(qc + 1) * NC],
                    start=(d == 0), stop=(d == DT - 1),
                )
            dst = ctxT[vt][:, qc * NC:(qc + 1) * NC]
            nc.vector.tensor_copy(dst, ps[:])

    if upto == "C2":
        _keepalive(
            nc, tc,
            [t[:, 0:1] for t in ctxT] + [t[:, 0:1] for t in wo_sb]
            + [gamma_sb[:, 0:1], beta_sb[:, 0:1]], out)
        stat_pool.release()
        h_pool.release()
        gb_pool.release()
        wo_pool.release()
        wv_pool.release()
        zt_pool.release()
        ctxT_pool.release()
        ps_b.release()
        return

    # ---- Phase O: h = ctx @ Wo, layernorm via accum_out + ln/exp rsqrt ----
    o_done = []
    for qt in range(QTN):
        h = h_pool.tile([P, D], F32, tag="h", name=f"h{qt}")
        sums = stat_pool.tile([P, 2], F32, tag="sums", name=f"sums{qt}")
        sqs = stat_pool.tile([P, 2], F32, tag="sqs", name=f"sqs{qt}")
        for dc in range(DCN):
            ps = ps_b.tile([P, NC], F32, tag="mm", name=f"psO{qt}_{dc}")
            for v in range(DT):
                nc.tensor.matmul(
                    ps[:], ctxT[v][:, qt * P:(qt + 1) * P],
                    wo_sb[v][:, dc * NC:(dc + 1) * NC],
                    start=(v == 0), stop=(v == DT - 1),
                )
            nc.scalar.activation(
                h[:, dc * NC:(dc + 1) * NC], ps[:], AF.Copy,
                accum_out=sums[:, dc:dc + 1],
            )
            nc.scalar.activation(
                sq_scr[:], h[:, dc * NC:(dc + 1) * NC], AF.Square,
                accum_out=sqs[:, dc:dc + 1],
            )
        mu = stat_pool.tile([P, 1], F32, tag="mu", name=f"mu{qt}")
        nc.vector.tensor_tensor(mu[:], sums[:, 0:1], sums[:, 1:2], OP.add)
        nc.vector.tensor_scalar(out=mu[:], in0=mu[:], scalar1=RD, scalar2=None, op0=OP.mult)
        m2 = stat_pool.tile([P, 1], F32, tag="m2", name=f"m2{qt}")
        nc.vector.tensor_tensor(m2[:], sqs[:, 0:1], sqs[:, 1:2], OP.add)
        nc.vector.tensor_scalar(out=m2[:], in0=m2[:], scalar1=RD, scalar2=None, op0=OP.mult)
        var = stat_pool.tile([P, 1], F32, tag="var", name=f"var{qt}")
        nc.vector.tensor_tensor(var[:], mu[:], mu[:], OP.mult)
        nc.vector.tensor_tensor(var[:], m2[:], var[:], OP.subtract)
        # rstd = exp(-0.5 * ln(var + eps)) — stays on the exp/ln act table
        lnv = stat_pool.tile([P, 1], F32, tag="lnv", name=f"lnv{qt}")
        nc.scalar.activation(lnv[:], var[:], AF.Ln, bias=eps_sb[:], scale=1.0)
        rstd = stat_pool.tile([P, 1], F32, tag="rstd", name=f"rstd{qt}")
        nc.scalar.activation(rstd[:], lnv[:], AF.Exp, scale=-0.5)
        # (h - mu)*gamma on DVE, then (*rstd) + beta on Pool, both in-place
        # in h: each engine appears once per tile so chains pipeline.
        nc.vector.scalar_tensor_tensor(
            out=h[:], in0=h[:], scalar=mu[:], in1=gamma_sb[:],
            op0=OP.subtract, op1=OP.mult,
        )
        nc.vector.scalar_tensor_tensor(
            out=h[:], in0=h[:], scalar=rstd[:], in1=beta_sb[:],
            op0=OP.mult, op1=OP.add,
        )
        o_done.append(h)
        if qt >= 2:
            q0 = qt - 2
            nc.scalar.dma_start(out[q0 * P:(q0 + 1) * P, :], o_done[q0][:])
    for q0 in (QTN - 2, QTN - 1):
        nc.scalar.dma_start(out[q0 * P:(q0 + 1) * P, :], o_done[q0][:])
    stat_pool.release()
    h_pool.release()
    gb_pool.release()
    wo_pool.release()
    wv_pool.release()
    zt_pool.release()
    ctxT_pool.release()
    ps_b.release()


_PROGS: dict = {}

_ACT_KEEP = "natural_log_exp_and_others"


def _compile_single_act_table(nc):
    """Compile with every act-table set except _ACT_KEEP hidden, so the
    table-load pass places exactly one LoadActFuncSet (ids stay valid
    because the list order/indexes are unchanged)."""
    orig = bacc.get_activation_tables

    def filtered(arch):
        full = orig(arch)
        if _ACT_KEEP not in full:
            return full
        return {name: (funcs if name == _ACT_KEEP else set())
                for name, funcs in full.items()}

    bacc.get_activation_tables = filtered
    try:
        nc.compile()
    finally:
        bacc.get_activation_tables = orig


def _build_program(n_iters: int = 1, upto: str = "full"):
    key = (n_iters, upto)
    if key not in _PROGS:
        nc = bacc.Bacc(
            "TRN2",
            target_bir_lowering=False,
            debug=False,
            enable_asserts=False,
            num_devices=N_CORES,
        )
        io = {
            "xb": nc.dram_tensor("xb", [S, D], F32R, kind="ExternalInput").ap(),
            "xt": nc.dram_tensor("xt", [D, S], F32R, kind="ExternalInput").ap(),
            "wq": nc.dram_tensor("wq", [D, D], F32R, kind="ExternalInput").ap(),
            "wk": nc.dram_tensor("wk", [D, D], F32R, kind="ExternalInput").ap(),
            "wv": nc.dram_tensor("wv", [D, D], F32R, kind="ExternalInput").ap(),
            "wo": nc.dram_tensor("wo", [D, D], F32R, kind="ExternalInput").ap(),
            "gamma_b": nc.dram_tensor("gamma_b", [P, D], F32, kind="ExternalInput").ap(),
            "beta_b": nc.dram_tensor("beta_b", [P, D], F32, kind="ExternalInput").ap(),
            "out": nc.dram_tensor("out", [NQ, D], F32, kind="ExternalOutput").ap(),
        }
        with tile.TileContext(nc) as tc:
            for _ in range(n_iters):
                with ExitStack() as ctx:
                    _emit(ctx, tc, io, upto)
        _compile_single_act_table(nc)
        _PROGS[key] = nc
    return _PROGS[key]


LAST_RESULTS = None


def kernel(x, Wq, Wk, Wv, Wo, ln2_gamma, ln2_beta):
    global LAST_RESULTS
    x = np.ascontiguousarray(np.asarray(x, dtype=np.float32))
    Wq = np.ascontiguousarray(np.asarray(Wq, dtype=np.float32))
    Wk = np.ascontiguousarray(np.asarray(Wk, dtype=np.float32))
    Wv = np.ascontiguousarray(np.asarray(Wv, dtype=np.float32))
    Wo = np.ascontiguousarray(np.asarray(Wo, dtype=np.float32))
    gamma_b = np.ascontiguousarray(
        np.broadcast_to(np.asarray(ln2_gamma, dtype=np.float32), (P, D))
    )
    beta_b = np.ascontiguousarray(
        np.broadcast_to(np.asarray(ln2_beta, dtype=np.float32), (P, D))
    )

    nc = _build_program()
    in_maps = []
    for c in range(N_CORES):
        b, h = c // 2, c % 2
        xr = np.ascontiguousarray(np.roll(x[b], -h * NQ, axis=0))
        in_maps.append(
            {
                "xb": xr,
                "xt": np.ascontiguousarray(xr.T),
                "wq": Wq,
                "wk": Wk,
                "wv": Wv,
                "wo": Wo,
                "gamma_b": gamma_b,
                "beta_b": beta_b,
            }
        )
    res = run_bass_kernel_spmd(nc, in_maps, list(range(N_CORES)))
    LAST_RESULTS = res
    out = np.empty((B, S, D), dtype=np.float32)
    for c in range(N_CORES):
        b, h = c // 2, c % 2
        out[b, h * NQ:(h + 1) * NQ] = res.results[c]["out"]
    return out


# revision 6
# speedup vs baseline: 1.1429x; 1.0435x over previous
"""Trainium2 Bass kernel v2: single-head attention encoder block.

x[4, 2048, 1024]; q/k/v projections, softmax attention, output projection,
layernorm.  8 NeuronCores, SPMD: core c = (batch c//2, query-half c%2).
Host supplies each core's x ROLLED (so its queries are rows 0:1024) in BOTH
layouts: xb [S, D] and xt = xb^T [D, S] — no on-device transposes.

Per-core dataflow (fp32r matmuls except the bf16 value path):
  K^T  = Wk^T @ x          ([k, s], 8-bank PSUM accumulation, d-outer)
  Q^T  = Wq^T @ x[:1024]   ([k, q], wq streamed in k-halves)
  S^T  = K Q^T -> exp      ([s, q] bf16 UNNORMALIZED attn; the softmax
                            denominator is never computed — LayerNorm is
                            invariant to positive row scaling, so it cancels
                            (eps mismatch costs ~2e-3 relative))
  Z^T  = x^T @ exp(S^T)    ([d, q]; bf16 x tiles staged + Pool cast)
  ctxT = Wv^T @ Z^T        ([v, q], unnormalized)
  h    = ctx @ Wo          ([q, d])
  out  = layernorm(h)*gamma + beta

LayerNorm touches ONLY the natural_log_exp activation table (exp/ln/copy/
square): sums come free from activation accum_out during the h drain, and
rstd = exp(-0.5*ln(var+eps)) — no Sqrt, so no act-table swaps anywhere.

Two independent HWDGE queues: SP carries weights/xt01/output, ACT carries
xt23 + the xb value stream, so streams never queue behind weight loads.
Epilogue work is spread across ACT (drains+stats), DVE (apply), Pool
(gamma/beta) so no engine exceeds the PE time per tile.
"""

from contextlib import ExitStack

import numpy as np

import concourse.bass as bass
import concourse.tile as tile
from concourse import bacc, mybir
from concourse.bass_utils import run_bass_kernel_spmd

F32 = mybir.dt.float32
F32R = mybir.dt.float32r
BF16 = mybir.dt.bfloat16
AF = mybir.ActivationFunctionType
OP = mybir.AluOpType

B = 4
S = 2048
D = 1024
NQ = 1024
P = 128
DT = D // P    # 8
ST = S // P    # 16
KTN = D // P   # 8
QTN = NQ // P  # 8
NC = 512
SCN = S // NC   # 4
QCN = NQ // NC  # 2
DCN = D // NC   # 2
N_CORES = 8
SCALE = 1.0 / np.sqrt(np.float32(D))
LN_EPS = 1e-5
RD = float(1.0 / D)


def _f32(ap):
    return ap.bitcast(F32)


def _keepalive(nc, tc, aps, out):
    kp = tc.alloc_tile_pool(name="keep", bufs=1, side="left")
    kt = kp.tile([P, max(len(aps), 1)], F32, tag="keep", name="keept")
    for i, ap in enumerate(aps):
        src_ap = ap[:, 0:1]
        if src_ap.dtype == F32R:
            src_ap = src_ap.bitcast(F32)
        nc.vector.tensor_copy(kt[:, i:i + 1], src_ap)
    nc.sync.dma_start(out[0:P, 0:max(len(aps), 1)], kt[:])
    kp.release()


def _emit(ctx: ExitStack, tc: tile.TileContext, io: dict, upto: str = "full"):
    nc = tc.nc
    xb = io["xb"]          # [S, D] f32r
    xt = io["xt"]          # [D, S] f32r
    wq = io["wq"]
    wk = io["wk"]
    wv = io["wv"]
    wo = io["wo"]
    gamma_b = io["gamma_b"]
    beta_b = io["beta_b"]
    out = io["out"]

    const = ctx.enter_context(tc.tile_pool(name="const", bufs=1, side="left"))
    eps_sb = const.tile([P, 1], F32, tag="eps")
    nc.vector.memset(eps_sb[:], LN_EPS)
    sq_scr = const.tile([P, NC], F32, tag="sqscr")

    # ---- SBUF stacks ----
    # left:  const | xt01 | wqs | wk | xt23, then S/C1: at|stage|xq0|xq1|den,
    #        then C2/O: wo | gb | h | o | stats
    # right: kt | qt, then ctxT | zt | wv
    xt01_pool = tc.alloc_tile_pool(name="xt01", bufs=1, side="left")
    xt01_all = xt01_pool.tile([P, DT * NQ], F32R, tag="xt01", name="xt01_all")
    xt01_v = [xt01_all[:, d * NQ:(d + 1) * NQ] for d in range(DT)]
    wqs_pool = tc.alloc_tile_pool(name="wqs", bufs=3, side="left")
    wk_pool = tc.alloc_tile_pool(name="wkp", bufs=1, side="left")
    wk_all = wk_pool.tile([P, DT * D], F32R, tag="wk", name="wk_all")
    wk_v = [wk_all[:, d * D:(d + 1) * D] for d in range(DT)]
    xt23_pool = tc.alloc_tile_pool(name="xt23", bufs=1, side="left")
    kt_pool = tc.alloc_tile_pool(name="ktp", bufs=1, side="right")
    kt_sb = [kt_pool.tile([P, S], F32R, tag=f"kt{k}", name=f"kt{k}") for k in range(KTN)]
    qt_pool = tc.alloc_tile_pool(name="qtp", bufs=1, side="right")
    qt_sb = [qt_pool.tile([P, NQ], F32R, tag=f"qt{k}", name=f"qt{k}") for k in range(KTN)]

    ps_a = tc.alloc_tile_pool(name="psA", bufs=1, space="PSUM")

    # DMA order.  SP queue carries every input (SP.SEQ runs ahead of the
    # compute engines, so next-iteration inputs start landing as soon as
    # their SBUF region frees at C1 end).  ACT queue carries only the
    # output tiles, so end-of-iteration waits never block SP.SEQ.
    # interleaved per-d DMAs: each (wk[d], xt01[d]) pair unlocks the next
    # d-step of the K phase, so PE starts after ~1 MB instead of 8 MB
    for d in range(DT):
        nc.sync.dma_start(wk_v[d], wk[d * P:(d + 1) * P, :])
        nc.sync.dma_start(xt01_v[d], xt[d * P:(d + 1) * P, 0:NQ])

    # ---- Phase K: K^T = Wk^T @ x, 8 accumulating banks, d streams ----
    xt23_r = {}
    for sc in (2, 3):
        for d in range(DT):
            t = xt23_pool.tile([P, NC], F32R, tag=f"x23_{d}", name=f"x23_{d}_{sc}")
            nc.sync.dma_start(t[:], xt[d * P:(d + 1) * P, sc * NC:(sc + 1) * NC])
            xt23_r[(sc, d)] = t
    rnd = 0
    for sc in range(SCN):
        for kh in range(2):
            st_set = rnd % 2
            rnd += 1
            ps = [ps_a.tile([P, NC], F32, tag=f"bank{st_set * 4 + kk}",
                            name=f"psK{kh * 4 + kk}_{sc}") for kk in range(4)]
            for d in range(DT):
                src = (xt01_all[:, d * NQ + sc * NC:d * NQ + (sc + 1) * NC] if sc < 2
                       else xt23_r[(sc, d)][:])
                for kk in range(4):
                    k = kh * 4 + kk
                    nc.tensor.matmul(
                        ps[kk][:], wk_all[:, d * D + k * P:d * D + (k + 1) * P], src,
                        start=(d == 0), stop=(d == DT - 1),
                    )
            for kk in range(4):
                k = kh * 4 + kk
                dst = kt_sb[k][:, sc * NC:(sc + 1) * NC]
                if kk % 2 == 0:
                    nc.vector.tensor_copy(dst, ps[kk][:])
                else:
                    nc.scalar.copy(dst, ps[kk][:])
    xt23_pool.release()
    wk_pool.release()

    if upto == "K":
        _keepalive(nc, tc, [t[:, 0:1] for t in kt_sb], out)
        ps_a.release()
        qt_pool.release()
        kt_pool.release()
        wqs_pool.release()
        xt01_pool.release()
        return

    # ---- Phase Q: Q^T = Wq^T @ x[:, :NQ], wq streamed in k-halves ----
    for kh in range(2):
        wq_h = []
        for d in range(DT):
            t = wqs_pool.tile([P, NC], F32R, tag="wqh", name=f"wqh{kh}_{d}")
            nc.sync.dma_start(t[:], wq[d * P:(d + 1) * P, kh * NC:(kh + 1) * NC])
            wq_h.append(t)
        for qc in range(QCN):
            st_set = rnd % 2
            rnd += 1
            ps = [ps_a.tile([P, NC], F32, tag=f"bank{st_set * 4 + kk}",
                            name=f"psQ{kh}_{qc}_{kk}") for kk in range(4)]
            for d in range(DT):
                for kk in range(4):
                    nc.tensor.matmul(
                        ps[kk][:],
                        wq_h[d][:, kk * P:(kk + 1) * P],
                        xt01_all[:, d * NQ + qc * NC:d * NQ + (qc + 1) * NC],
                        start=(d == 0), stop=(d == DT - 1),
                    )
            for kk in range(4):
                k = kh * 4 + kk
                dst = qt_sb[k][:, qc * NC:(qc + 1) * NC]
                if kk % 2 == 0:
                    nc.vector.tensor_copy(dst, ps[kk][:])
                else:
                    nc.scalar.copy(dst, ps[kk][:])
    wqs_pool.release()
    xt01_pool.release()
    ps_a.release()

    if upto == "Q":
        _keepalive(nc, tc, [t[:, 0:1] for t in kt_sb] + [t[:, 0:1] for t in qt_sb], out)
        qt_pool.release()
        kt_pool.release()
        return

    # ---- Phase S: scores^T -> exp -> bf16 at; xb value stream starts ----
    at_pool = tc.alloc_tile_pool(name="atp", bufs=1, side="left")
    at_sb = [at_pool.tile([P, NQ], BF16, tag=f"at{st}", name=f"at{st}") for st in range(ST)]
    stage_pool = tc.alloc_tile_pool(name="stage", bufs=6, side="left")
    xq0_pool = tc.alloc_tile_pool(name="xq0", bufs=1, side="left")
    xq0 = [xq0_pool.tile([P, NC], BF16, tag=f"xq0_{st}", name=f"xq0_{st}")
           for st in range(ST)]
    xq1_pool = tc.alloc_tile_pool(name="xq1", bufs=1, side="left")
    xq1 = [xq1_pool.tile([P, NC], BF16, tag=f"xq1_{st}", name=f"xq1_{st}")
           for st in range(ST)]
    ps_b = tc.alloc_tile_pool(name="psB", bufs=7, space="PSUM")

    # xb stream: DMA fp32 -> stage (ACT queue), Pool casts to bf16 tiles.
    for dp in range(2):
        for st in range(ST):
            stg = stage_pool.tile([P, NC], F32R, tag="stg", name=f"stg{dp}_{st}")
            nc.sync.dma_start(stg[:], xb[st * P:(st + 1) * P, dp * NC:(dp + 1) * NC])
            dst = (xq0 if dp == 0 else xq1)[st]
            nc.gpsimd.tensor_copy(dst[:], _f32(stg[:]))

    for qc in range(QCN):
        for st in range(ST):
            ps = ps_b.tile([P, NC], F32, tag="mm", name=f"psS{qc}_{st}")
            for k in range(KTN):
                nc.tensor.matmul(
                    ps[:], kt_sb[k][:, st * P:(st + 1) * P],
                    qt_sb[k][:, qc * NC:(qc + 1) * NC],
                    start=(k == 0), stop=(k == KTN - 1),
                )
            nc.scalar.activation(
                at_sb[st][:, qc * NC:(qc + 1) * NC], ps[:], AF.Exp, scale=float(SCALE)
            )
    qt_pool.release()
    kt_pool.release()

    if upto == "S":
        _keepalive(nc, tc, [t[:, 0:1] for t in at_sb], out)
        xq1_pool.release()
        xq0_pool.release()
        stage_pool.release()
        at_pool.release()
        ps_b.release()
        return

    # ---- Phase C1: Z^T = x^T @ exp(S^T), bf16 matmuls ----
    # The whole value path (at/xq/zt/wv/ctxT/wo) is bf16: same PE rate, and
    # the halved right-stack footprint keeps it clear of the next
    # iteration's left-stack input region, so input DMAs overlap C2/O.
    ctxT_pool = tc.alloc_tile_pool(name="ctxTp", bufs=1, side="right")
    ctxT = [ctxT_pool.tile([P, NQ], BF16, tag=f"cxT{v}", name=f"cxT{v}") for v in range(DT)]
    zt_pool = tc.alloc_tile_pool(name="ztp", bufs=1, side="right")
    zt_sb = [zt_pool.tile([P, NQ], BF16, tag=f"zt{d}", name=f"zt{d}") for d in range(DT)]
    wv_pool = tc.alloc_tile_pool(name="wvp", bufs=1, side="right")
    wv_sb = [wv_pool.tile([P, D], BF16, tag=f"wv{d}", name=f"wv{d}") for d in range(DT)]
    wstage_pool = tc.alloc_tile_pool(name="wstage", bufs=2, side="left")
    for d in range(DT):
        ws = wstage_pool.tile([P, D], F32R, tag="ws", name=f"wvs{d}")
        nc.sync.dma_start(ws[:], wv[d * P:(d + 1) * P, :])
        nc.gpsimd.tensor_copy(wv_sb[d][:], _f32(ws[:]))

    for dp in range(2):
        xq = xq0 if dp == 0 else xq1
        for dh in range(4):
            d = 4 * dp + dh
            for qc in range(QCN):
                ps = ps_b.tile([P, NC], F32, tag="mm", name=f"psZ{d}_{qc}")
                for st in range(ST):
                    nc.tensor.matmul(
                        ps[:], xq[st][:, dh * P:(dh + 1) * P],
                        at_sb[st][:, qc * NC:(qc + 1) * NC],
                        start=(st == 0), stop=(st == ST - 1),
                    )
                dst = zt_sb[d][:, qc * NC:(qc + 1) * NC]
                if d % 2 == 0:
                    nc.vector.tensor_copy(dst, ps[:])
                else:
                    nc.scalar.copy(dst, ps[:])
    wstage_pool.release()
    xq1_pool.release()
    xq0_pool.release()
    stage_pool.release()
    at_pool.release()

    if upto == "C1":
        _keepalive(
            nc, tc,
            [t[:, 0:1] for t in zt_sb] + [t[:, 0:1] for t in wv_sb],
            out)
        wv_pool.release()
        zt_pool.release()
        ctxT_pool.release()
        ps_b.release()
        return

    # ---- Phase C2: ctxT = Wv^T @ Z^T (unnormalized) ----
    wo_pool = tc.alloc_tile_pool(name="wop", bufs=1, side="right")
    wo_sb = [wo_pool.tile([P, D], BF16, tag=f"wo{v}", name=f"wo{v}") for v in range(DT)]
    gb_pool = tc.alloc_tile_pool(name="gbp", bufs=1, side="right")
    gamma_sb = gb_pool.tile([P, D], F32, tag="gamma", name="gamma_sb")
    nc.sync.dma_start(gamma_sb[:], gamma_b[:])
    beta_sb = gb_pool.tile([P, D], F32, tag="beta", name="beta_sb")
    nc.sync.dma_start(beta_sb[:], beta_b[:])
    h_pool = tc.alloc_tile_pool(name="hp", bufs=3, side="right")
    stat_pool = tc.alloc_tile_pool(name="statp", bufs=4, side="right")
    # wo loads staged through h tiles (fp32), cast to bf16 by Pool
    for v in range(DT):
        ws = h_pool.tile([P, D], F32, tag="h", name=f"wos{v}")
        nc.sync.dma_start(ws[:].bitcast(F32R), wo[v * P:(v + 1) * P, :])
        nc.gpsimd.tensor_copy(wo_sb[v][:], ws[:])

    for vt in range(DT):
        for qc in range(QCN):
            ps = ps_b.tile([P, NC], F32, tag="mm", name=f"psC{vt}_{qc}")
            for d in range(DT):
                nc.tensor.matmul(
                    ps[:], wv_sb[d][:, vt * P:(vt + 1) * P],
                    zt_sb[d][:, qc * NC:(qc + 1) * NC],
                    start=(d == 0), stop=(d == DT - 1),
                )
            dst = ctxT[vt][:, qc * NC:(qc + 1) * NC]
            nc.vector.tensor_copy(dst, ps[:])

    if upto == "C2":
        _keepalive(
            nc, tc,
            [t[:, 0:1] for t in ctxT] + [t[:, 0:1] for t in wo_sb]
            + [gamma_sb[:, 0:1], beta_sb[:, 0:1]], out)
        stat_pool.release()
        h_pool.release()
        gb_pool.release()
        wo_pool.release()
        wv_pool.release()
        zt_pool.release()
        ctxT_pool.release()
        ps_b.release()
        return

    # ---- Phase O: h = ctx @ Wo, layernorm via accum_out + ln/exp rsqrt ----
    o_done = []
    for qt in range(QTN):
        h = h_pool.tile([P, D], F32, tag="h", name=f"h{qt}")
        sums = stat_pool.tile([P, 2], F32, tag="sums", name=f"sums{qt}")
        sqs = stat_pool.tile([P, 2], F32, tag="sqs", name=f"sqs{qt}")
        for dc in range(DCN):
            ps = ps_b.tile([P, NC], F32, tag="mm", name=f"psO{qt}_{dc}")
            for v in range(DT):
                nc.tensor.matmul(
                    ps[:], ctxT[v][:, qt * P:(qt + 1) * P],
                    wo_sb[v][:, dc * NC:(dc + 1) * NC],
                    start=(v == 0), stop=(v == DT - 1),
                )
            nc.scalar.activation(
                h[:, dc * NC:(dc + 1) * NC], ps[:], AF.Copy,
                accum_out=sums[:, dc:dc + 1],
            )
            nc.scalar.activation(
                sq_scr[:], h[:, dc * NC:(dc + 1) * NC], AF.Square,
                accum_out=sqs[:, dc:dc + 1],
            )
        mu = stat_pool.tile([P, 1], F32, tag="mu", name=f"mu{qt}")
        nc.vector.tensor_tensor(mu[:], sums[:, 0:1], sums[:, 1:2], OP.add)
        nc.vector.tensor_scalar(out=mu[:], in0=mu[:], scalar1=RD, scalar2=None, op0=OP.mult)
        m2 = stat_pool.tile([P, 1], F32, tag="m2", name=f"m2{qt}")
        nc.vector.tensor_tensor(m2[:], sqs[:, 0:1], sqs[:, 1:2], OP.add)
        nc.vector.tensor_scalar(out=m2[:], in0=m2[:], scalar1=RD, scalar2=None, op0=OP.mult)
        var = stat_pool.tile([P, 1], F32, tag="var", name=f"var{qt}")
        nc.vector.tensor_tensor(var[:], mu[:], mu[:], OP.mult)
        nc.vector.tensor_tensor(var[:], m2[:], var[:], OP.subtract)
        # rstd = exp(-0.5 * ln(var + eps)) — stays on the exp/ln act table
        lnv = stat_pool.tile([P, 1], F32, tag="lnv", name=f"lnv{qt}")
        nc.scalar.activation(lnv[:], var[:], AF.Ln, bias=eps_sb[:], scale=1.0)
        rstd = stat_pool.tile([P, 1], F32, tag="rstd", name=f"rstd{qt}")
        nc.scalar.activation(rstd[:], lnv[:], AF.Exp, scale=-0.5)
        # (h - mu)*gamma on DVE, then (*rstd) + beta on Pool, both in-place
        # in h: each engine appears once per tile so chains pipeline.
        nc.vector.scalar_tensor_tensor(
            out=h[:], in0=h[:], scalar=mu[:], in1=gamma_sb[:],
            op0=OP.subtract, op1=OP.mult,
        )
        nc.vector.scalar_tensor_tensor(
            out=h[:], in0=h[:], scalar=rstd[:], in1=beta_sb[:],
            op0=OP.mult, op1=OP.add,
        )
        o_done.append(h)
        if qt >= 2:
            q0 = qt - 2
            nc.scalar.dma_start(out[q0 * P:(q0 + 1) * P, :], o_done[q0][:])
    for q0 in (QTN - 2, QTN - 1):
        nc.scalar.dma_start(out[q0 * P:(q0 + 1) * P, :], o_done[q0][:])
    stat_pool.release()
    h_pool.release()
    gb_pool.release()
    wo_pool.release()
    wv_pool.release()
    zt_pool.release()
    ctxT_pool.release()
    ps_b.release()


_PROGS: dict = {}

_ACT_KEEP = "natural_log_exp_and_others"


def _compile_single_act_table(nc):
    """Compile with every act-table set except _ACT_KEEP hidden, so the
    table-load pass places exactly one LoadActFuncSet (ids stay valid
    because the list order/indexes are unchanged)."""
    orig = bacc.get_activation_tables

    def filtered(arch):
        full = orig(arch)
        if _ACT_KEEP not in full:
            return full
        return {name: (funcs if name == _ACT_KEEP else set())
                for name, funcs in full.items()}

    bacc.get_activation_tables = filtered
    try:
        nc.compile()
    finally:
        bacc.get_activation_tables = orig


def _build_program(n_iters: int = 1, upto: str = "full"):
    key = (n_iters, upto)
    if key not in _PROGS:
        nc = bacc.Bacc(
            "TRN2",
            target_bir_lowering=False,
            debug=False,
            enable_asserts=False,
            num_devices=N_CORES,
        )
        io = {
            "xb": nc.dram_tensor("xb", [S, D], F32R, kind="ExternalInput").ap(),
            "xt": nc.dram_tensor("xt", [D, S], F32R, kind="ExternalInput").ap(),
            "wq": nc.dram_tensor("wq", [D, D], F32R, kind="ExternalInput").ap(),
            "wk": nc.dram_tensor("wk", [D, D], F32R, kind="ExternalInput").ap(),
            "wv": nc.dram_tensor("wv", [D, D], F32R, kind="ExternalInput").ap(),
            "wo": nc.dram_tensor("wo", [D, D], F32R, kind="ExternalInput").ap(),
            "gamma_b": nc.dram_tensor("gamma_b", [P, D], F32, kind="ExternalInput").ap(),
            "beta_b": nc.dram_tensor("beta_b", [P, D], F32, kind="ExternalInput").ap(),
            "out": nc.dram_tensor("out", [NQ, D], F32, kind="ExternalOutput").ap(),
        }
        with tile.TileContext(nc) as tc:
            for _ in range(n_iters):
                with ExitStack() as ctx:
                    _emit(ctx, tc, io, upto)
        _compile_single_act_table(nc)
        _PROGS[key] = nc
    return _PROGS[key]


LAST_RESULTS = None


def kernel(x, Wq, Wk, Wv, Wo, ln2_gamma, ln2_beta):
    global LAST_RESULTS
    x = np.ascontiguousarray(np.asarray(x, dtype=np.float32))
    Wq = np.ascontiguousarray(np.asarray(Wq, dtype=np.float32))
    Wk = np.ascontiguousarray(np.asarray(Wk, dtype=np.float32))
    Wv = np.ascontiguousarray(np.asarray(Wv, dtype=np.float32))
    Wo = np.ascontiguousarray(np.asarray(Wo, dtype=np.float32))
    gamma_b = np.ascontiguousarray(
        np.broadcast_to(np.asarray(ln2_gamma, dtype=np.float32), (P, D))
    )
    beta_b = np.ascontiguousarray(
        np.broadcast_to(np.asarray(ln2_beta, dtype=np.float32), (P, D))
    )

    nc = _build_program()
    in_maps = []
    for c in range(N_CORES):
        b, h = c // 2, c % 2
        xr = np.ascontiguousarray(np.roll(x[b], -h * NQ, axis=0))
        in_maps.append(
            {
                "xb": xr,
                "xt": np.ascontiguousarray(xr.T),
                "wq": Wq,
                "wk": Wk,
                "wv": Wv,
                "wo": Wo,
                "gamma_b": gamma_b,
                "beta_b": beta_b,
            }
        )
    res = run_bass_kernel_spmd(nc, in_maps, list(range(N_CORES)))
    LAST_RESULTS = res
    out = np.empty((B, S, D), dtype=np.float32)
    for c in range(N_CORES):
        b, h = c // 2, c % 2
        out[b, h * NQ:(h + 1) * NQ] = res.results[c]["out"]
    return out
